# revision 19
# baseline (speedup 1.0000x reference)
"""Bahdanau-attention kernel for Trainium2, 8-core data-parallel over batch.

Problem: context = softmax(w2 . tanh(enc @ W1_enc + hid @ W1_hid + b1)) @ enc
  B=32, S=2048, D=1024.  Each of the 8 cores handles 4 batch elements.

Device-side strategy (per core, per batch b, per 512-wide seq chunk):
  - encT [D, S] (host-transposed, bf16) slices feed the big matmul
    h^T[m-chunk] = sum_k W1_enc[k,m]^T @ encT[k]   (PE, PSUM f32 accum)
  - W1_enc is fp8-e3m4 (x128 host prescale): every matmul's LDWEIGHTS is
    serialized with the matmul on this toolchain, and FWL loads fp8 weights
    at 4 B/cycle, so fp8 halves the per-matmul weight-load tax vs bf16.
    e3m4 keeps 4 mantissa bits (~1.2% quant err); the x128 prescale keeps
    values out of the subnormal range and is undone by the tanh's scale.
  - tanh+bias via ACT: h = tanh(hp/128 + z), z[m] = (hid @ W1_hid + b1)[m]
  - e-scores in row form: e_row[1, 512] = sum_m w2[m]^T @ h^T[m]  (PE),
    emitted one m-step behind the h matmuls so the PE never waits on ACT
  - p_row = exp(e_row) on ACT with fused accum_out giving the softmax
    normalizer partial (no max subtraction: |e| <= sum|w2| ~ 26, safe fp32)
  - p broadcast to 128 partitions via K=1 ones-matmul on PE; exp/broadcast/
    context work for chunk i is emitted inside chunk i+1's matmul stream so
    the PE pipeline stays dense
  - ctx contribution on the otherwise-idle DVE: one broadcast multiply +
    one grouped reduce over the already-resident encT tile (the natural
    layout enc copy is never loaded; halves HBM traffic)
  - finalize once per invocation, off the critical path: per-batch Z sums
    land in one [1, BL] tile; 1/Z = exp(-ln Z) on ACT (vector.reciprocal
    costs ~7us on DVE and stalls the DVE FIFO); one contiguous [P, BL*KC]
    store (the old per-batch [p,k]->(k p) store was a 1024-descriptor
    strided DMA), host permutes the output back to [BL, D].
"""

import numpy as np
from contextlib import ExitStack

import concourse.bacc as bacc
import concourse.tile as tile
from concourse import mybir
from concourse.bass_utils import run_bass_kernel_spmd

AFT = mybir.ActivationFunctionType
ALU = mybir.AluOpType
F32 = mybir.dt.float32

B, S, D = 32, 2048, 1024
NCORES = 8
BL = B // NCORES          # 4 batch elements per core
P = 128
KC = D // P               # 8 contraction / output chunks
S_SUB = 512               # seq chunk processed per inner iteration
NSS = S // S_SUB          # 4

# moving-side dtype (activations, enc) and stationary weight dtype
DT = mybir.dt.bfloat16
WT = mybir.dt.float8e3    # e3m4, 4 mantissa bits
W1E_SCALE = 128.0         # host prescale so W1_enc avoids e3m4 subnormals


def _body(ctx, tc, encT, hidT, w1e, w1h, b1, w2, out):
    nc = tc.nc
    # bufs=2 on the weight/const pools: in the For_i timing loop the next
    # rep's weight DMA then overlaps this rep's tail compute instead of
    # serializing behind the last matmul that reads the old weights.
    const = ctx.enter_context(tc.tile_pool(name="const", bufs=2))
    wpool = ctx.enter_context(tc.tile_pool(name="wpool", bufs=2))
    epool = ctx.enter_context(tc.tile_pool(name="epool", bufs=4))
    spool2 = ctx.enter_context(tc.tile_pool(name="spool2", bufs=2))
    hpool = ctx.enter_context(tc.tile_pool(name="hpool", bufs=4))
    spool = ctx.enter_context(tc.tile_pool(name="spool", bufs=2))
    cpool = ctx.enter_context(tc.tile_pool(name="cpool", bufs=2 * NSS + 2))
    fpool = ctx.enter_context(tc.tile_pool(name="fpool", bufs=2))
    pbpool = ctx.enter_context(tc.tile_pool(name="pbpool", bufs=2))
    # PSUM budget (8 banks): hp/zp 6 + e_row 2
    ppa = ctx.enter_context(tc.tile_pool(name="ppa", bufs=6, space="PSUM"))
    ppe = ctx.enter_context(tc.tile_pool(name="ppe", bufs=2, space="PSUM"))

    # --- phase 0 DMA order matters for startup: the z matmuls are first in
    # the PE stream, so their inputs (w1h, hidT) go first; then w1e
    # interleaved with the first chunk's encT slices; small consts last.
    w1e_t, w1h_t, hid_t, b1_t, w2_t = [], [], [], [], []
    for k in range(KC):
        t = wpool.tile([P, D], DT, name=f"w1h_{k}")
        nc.sync.dma_start(t[:], w1h[k * P:(k + 1) * P, :])
        w1h_t.append(t)
        t = const.tile([P, BL], DT, name=f"hid_{k}")
        nc.sync.dma_start(t[:], hidT[k * P:(k + 1) * P, :])
        hid_t.append(t)
    et0 = epool.tile([P, KC * S_SUB], DT, name="et_big", tag="et_big")
    for k in range(KC):
        t = wpool.tile([P, D], WT, name=f"w1e_{k}")
        nc.sync.dma_start(t[:], w1e[k * P:(k + 1) * P, :])
        w1e_t.append(t)
        nc.sync.dma_start(et0[:, k * S_SUB:(k + 1) * S_SUB],
                          encT[0, k * P:(k + 1) * P, 0:S_SUB])
    for k in range(KC):
        t = const.tile([P, 1], F32, name=f"b1_{k}")
        nc.sync.dma_start(t[:], b1[k * P:(k + 1) * P, :])
        b1_t.append(t)
        t = const.tile([P, 2], DT, name=f"w2_{k}")
        nc.sync.dma_start(t[:], w2[k * P:(k + 1) * P, :])
        w2_t.append(t)

    # per-batch bias z = hid @ W1_hid + b1
    z_sb = []
    for m in range(KC):
        zp = ppa.tile([P, BL], F32, name="zp", tag="ppa_t")
        for k in range(KC):
            nc.tensor.matmul(
                zp[:], lhsT=w1h_t[k][:, m * P:(m + 1) * P], rhs=hid_t[k][:],
                start=(k == 0), stop=(k == KC - 1))
        zt = const.tile([P, BL], F32, name=f"z_{m}")
        nc.vector.tensor_scalar_add(zt[:], zp[:], b1_t[m][:])
        z_sb.append(zt)

    # per-invocation accumulators for the deferred finalize
    zvec = fpool.tile([1, BL], F32, name="zvec")
    ctxall = fpool.tile([P, BL * KC], F32, name="ctxall")

    # --- pipelined main loop ---
    state = {}    # per-batch: z_parts tile + list of per-chunk ctx tiles
    pending = None  # chunk awaiting exp/broadcast/ctx emission

    def emit_post(pend):
        """exp, p-broadcast, and DVE context work for a finished chunk."""
        pb, pss, e_ps, et_big = pend
        st = state[pb]
        p_row = spool.tile([1, S_SUB], DT, name="p_row", tag="p_row")
        nc.scalar.activation(p_row[:], e_ps[0:1, :], AFT.Exp,
                             accum_out=st["z_parts"][0:1, pss:pss + 1])
        # broadcast p to 128 partitions on the otherwise-idle GPSIMD engine,
        # straight to bf16 SBUF so the DVE multiply+reduce run in the
        # all-16-bit all-SBUF 4x perf mode (PSUM f32 operands force 1x)
        pbc_sb = pbpool.tile([P, S_SUB], DT, name="pbc_sb", tag="pbc_sb")
        nc.gpsimd.partition_broadcast(pbc_sb[:], p_row[:])
        scratch = spool2.tile([P, KC * S_SUB], DT, name="scr", tag="scr")
        p_rep = pbc_sb[:].rearrange("p (o s) -> p o s",
                                    o=1).broadcast_to((P, KC, S_SUB))
        nc.vector.tensor_tensor(
            scratch[:].rearrange("p (k s) -> p k s", k=KC),
            et_big[:].rearrange("p (k s) -> p k s", k=KC),
            p_rep, ALU.mult)
        cred = cpool.tile([P, KC], DT, name="cred", tag="cred")
        with nc.allow_low_precision(reason="bf16 cred keeps the DVE reduce "
                                    "in 4x mode; ~0.4% on a 2e-2 budget"):
            nc.vector.tensor_reduce(
                cred[:], scratch[:].rearrange("p (k s) -> p k s", k=KC),
                axis=mybir.AxisListType.X, op=ALU.add)
        st["creds"].append(cred)
        if pss == NSS - 1:
            emit_batch_reduce(pb)

    def emit_batch_reduce(pb):
        """Per-batch: Z partial sum + unnormalized ctx into the rep-wide
        accumulators. Runs overlapped with the next batch's matmuls."""
        st = state.pop(pb)
        nc.vector.tensor_reduce(zvec[0:1, pb:pb + 1], st["z_parts"][:],
                                axis=mybir.AxisListType.X, op=ALU.add)
        creds = st["creds"]
        dst = ctxall[:, pb * KC:(pb + 1) * KC]
        nc.vector.tensor_tensor(dst, creds[0][:], creds[1][:], ALU.add)
        nc.vector.tensor_tensor(dst, dst, creds[2][:], ALU.add)
        nc.vector.tensor_tensor(dst, dst, creds[3][:], ALU.add)

    for ci, (b, ss) in enumerate([(b, ss) for b in range(BL)
                                  for ss in range(NSS)]):
        if ss == 0:
            state[b] = {
                "z_parts": spool.tile([1, NSS], F32, name="z_parts",
                                      tag="z_parts"),
                "creds": [],
            }
        if ci == 0:
            et_big = et0
        else:
            et_big = epool.tile([P, KC * S_SUB], DT, name="et_big",
                                tag="et_big")
            for k in range(KC):
                nc.sync.dma_start(
                    et_big[:, k * S_SUB:(k + 1) * S_SUB],
                    encT[b, k * P:(k + 1) * P, ss * S_SUB:(ss + 1) * S_SUB])
        e_ps = ppe.tile([2, S_SUB], F32, name="e_ps")
        h_prev = None
        for m in range(KC):
            hp = ppa.tile([P, S_SUB], F32, name="hp", tag="ppa_t")
            for k in range(KC):
                nc.tensor.matmul(
                    hp[:], lhsT=w1e_t[k][:, m * P:(m + 1) * P],
                    rhs=et_big[:, k * S_SUB:(k + 1) * S_SUB],
                    start=(k == 0), stop=(k == KC - 1))
            if m == 0 and pending is not None:
                emit_post(pending)
                pending = None
            h_sb = hpool.tile([P, S_SUB], DT, name="h_sb", tag="h_sb")
            nc.scalar.activation(h_sb[:], hp[:], AFT.Tanh,
                                 bias=z_sb[m][:, b:b + 1],
                                 scale=1.0 / W1E_SCALE)
            if h_prev is not None:
                nc.tensor.matmul(e_ps[:], lhsT=w2_t[m - 1][:], rhs=h_prev[:],
                                 start=(m == 1), stop=False)
            h_prev = h_sb
        nc.tensor.matmul(e_ps[:], lhsT=w2_t[KC - 1][:], rhs=h_prev[:],
                         start=False, stop=True)
        pending = (b, ss, e_ps, et_big)
    emit_post(pending)

    # --- deferred finalize: ctx / Z, one contiguous store ---
    zbc = spool.tile([P, BL], F32, name="zbc", tag="zbc")
    nc.gpsimd.partition_broadcast(zbc[:], zvec[:])
    lnz = spool.tile([P, BL], F32, name="lnz", tag="lnz")
    nc.scalar.activation(lnz[:], zbc[:], AFT.Ln)
    zr = spool.tile([P, BL], F32, name="zr", tag="zr")
    nc.scalar.activation(zr[:], lnz[:], AFT.Exp, scale=-1.0)
    ctxout = fpool.tile([P, BL * KC], F32, name="ctxout")
    for b in range(BL):
        nc.vector.tensor_scalar_mul(ctxout[:, b * KC:(b + 1) * KC],
                                    ctxall[:, b * KC:(b + 1) * KC],
                                    zr[:, b:b + 1])
    nc.sync.dma_start(out[:, :], ctxout[:])


def build_program():
    nc = bacc.Bacc("TRN2", target_bir_lowering=False, debug=False,
                   num_devices=NCORES)
    encT = nc.dram_tensor("encT", [BL, D, S], DT, kind="ExternalInput").ap()
    hidT = nc.dram_tensor("hidT", [D, BL], DT, kind="ExternalInput").ap()
    w1e = nc.dram_tensor("w1e", [D, D], WT, kind="ExternalInput").ap()
    w1h = nc.dram_tensor("w1h", [D, D], DT, kind="ExternalInput").ap()
    b1 = nc.dram_tensor("b1", [D, 1], F32, kind="ExternalInput").ap()
    w2 = nc.dram_tensor("w2", [D, 2], DT, kind="ExternalInput").ap()
    # ctx in [partition, batch*KC] layout; host permutes back to [BL, D]
    out = nc.dram_tensor("ctx_out", [P, BL * KC], F32,
                         kind="ExternalOutput").ap()
    with tile.TileContext(nc) as tc:
        with ExitStack() as ctx:
            _body(ctx, tc, encT, hidT, w1e, w1h, b1, w2, out)
    nc.compile()
    return nc


def prep_in_maps(inputs):
    import ml_dtypes
    bf16 = ml_dtypes.bfloat16
    fp8 = ml_dtypes.float8_e3m4
    enc = np.asarray(inputs["encoder_outputs"], dtype=np.float32)
    hid = np.asarray(inputs["hidden_state"], dtype=np.float32)
    W1 = np.asarray(inputs["W1"], dtype=np.float32)
    b1 = np.asarray(inputs["b1"], dtype=np.float32)
    w2 = np.asarray(inputs["w2"], dtype=np.float32)
    encT = np.ascontiguousarray(enc.transpose(0, 2, 1)).astype(bf16)
    w1e = np.clip(np.ascontiguousarray(W1[:D]) * W1E_SCALE,
                  -15.5, 15.5).astype(fp8)
    w1h = np.ascontiguousarray(W1[D:]).astype(bf16)
    b1c = np.ascontiguousarray(b1.reshape(D, 1))
    w2c = np.zeros((D, 2), dtype=bf16)
    w2c[:, 0] = w2.astype(bf16)
    in_maps = []
    for c in range(NCORES):
        sl = slice(c * BL, (c + 1) * BL)
        in_maps.append({
            "encT": encT[sl],
            "hidT": np.ascontiguousarray(hid[sl].T).astype(bf16),
            "w1e": w1e,
            "w1h": w1h,
            "b1": b1c,
            "w2": w2c,
        })
    return in_maps


_NC_CACHE = None


def unpack_out(arr):
    """Device layout [P, BL*KC] -> [BL, D]: [p, b*KC+k] = ctx[b, k*P+p]."""
    return (np.asarray(arr, dtype=np.float32).reshape(P, BL, KC)
            .transpose(1, 2, 0).reshape(BL, D))


def kernel(**inputs):
    global _NC_CACHE
    if _NC_CACHE is None:
        _NC_CACHE = build_program()
    nc = _NC_CACHE
    in_maps = prep_in_maps(inputs)
    res = run_bass_kernel_spmd(nc, in_maps, core_ids=list(range(NCORES)))
    out = np.empty((B, D), dtype=np.float32)
    for c in range(NCORES):
        out[c * BL:(c + 1) * BL] = unpack_out(res.results[c]["ctx_out"])
    return out


# revision 21
# speedup vs baseline: 1.1520x; 1.1520x over previous
"""Bahdanau-attention kernel for Trainium2, 8-core data-parallel over batch.

Problem: context = softmax(w2 . tanh(enc @ W1_enc + hid @ W1_hid + b1)) @ enc
  B=32, S=2048, D=1024.  Each of the 8 cores handles 4 batch elements.

Device-side strategy (per core, per batch b, per 512-wide seq chunk):
  - encT [D, S] (host-transposed, bf16) slices feed the big matmul
    h^T[m-chunk] = sum_k W1_enc[k,m]^T @ encT[k]   (PE, PSUM f32 accum)
  - W1_enc is fp8-e3m4 (x128 host prescale): every matmul's LDWEIGHTS is
    serialized with the matmul on this toolchain, and FWL loads fp8 weights
    at 4 B/cycle, so fp8 halves the per-matmul weight-load tax vs bf16.
    e3m4 keeps 4 mantissa bits (~1.2% quant err); the x128 prescale keeps
    values out of the subnormal range and is undone by the tanh's scale.
  - tanh+bias via ACT: h = tanh(hp/128 + z), z[m] = (hid @ W1_hid + b1)[m]
  - e-scores in row form: e_row[1, 512] = sum_m w2[m]^T @ h^T[m]  (PE),
    emitted one m-step behind the h matmuls so the PE never waits on ACT
  - p_row = exp(e_row) on ACT with fused accum_out giving the softmax
    normalizer partial (no max subtraction: |e| <= sum|w2| ~ 26, safe fp32)
  - p broadcast to 128 partitions via K=1 ones-matmul on PE; exp/broadcast/
    context work for chunk i is emitted inside chunk i+1's matmul stream so
    the PE pipeline stays dense
  - ctx contribution on the otherwise-idle DVE: one broadcast multiply +
    one grouped reduce over the already-resident encT tile (the natural
    layout enc copy is never loaded; halves HBM traffic)
  - finalize once per invocation, off the critical path: per-batch Z sums
    land in one [1, BL] tile; 1/Z = exp(-ln Z) on ACT (vector.reciprocal
    costs ~7us on DVE and stalls the DVE FIFO); one contiguous [P, BL*KC]
    store (the old per-batch [p,k]->(k p) store was a 1024-descriptor
    strided DMA), host permutes the output back to [BL, D].
"""

import numpy as np
from contextlib import ExitStack

import concourse.bacc as bacc
import concourse.tile as tile
from concourse import mybir
from concourse.bass_utils import run_bass_kernel_spmd

AFT = mybir.ActivationFunctionType
ALU = mybir.AluOpType
F32 = mybir.dt.float32

B, S, D = 32, 2048, 1024
NCORES = 8
BL = B // NCORES          # 4 batch elements per core
P = 128
KC = D // P               # 8 contraction / output chunks
S_SUB = 512               # seq chunk processed per inner iteration
NSS = S // S_SUB          # 4

# moving-side dtype (activations, enc) and stationary weight dtype
DT = mybir.dt.bfloat16
WT = mybir.dt.float8e3    # e3m4, 4 mantissa bits
W1E_SCALE = 128.0         # host prescale so W1_enc avoids e3m4 subnormals


def _body(ctx, tc, encT, hidT, w1e, w1h, b1, w2, onesr, out):
    nc = tc.nc
    # bufs=2 on the weight/const pools: in the For_i timing loop the next
    # rep's weight DMA then overlaps this rep's tail compute instead of
    # serializing behind the last matmul that reads the old weights.
    const = ctx.enter_context(tc.tile_pool(name="const", bufs=2))
    wpool = ctx.enter_context(tc.tile_pool(name="wpool", bufs=2))
    epool = ctx.enter_context(tc.tile_pool(name="epool", bufs=4))
    spool2 = ctx.enter_context(tc.tile_pool(name="spool2", bufs=2))
    hpool = ctx.enter_context(tc.tile_pool(name="hpool", bufs=4))
    spool = ctx.enter_context(tc.tile_pool(name="spool", bufs=2))
    cpool = ctx.enter_context(tc.tile_pool(name="cpool", bufs=2 * NSS + 2))
    fpool = ctx.enter_context(tc.tile_pool(name="fpool", bufs=2))
    pbpool = ctx.enter_context(tc.tile_pool(name="pbpool", bufs=2))
    # PSUM budget (8 banks): hp/zp 4 + e_row 2 + p_bc 2
    ppa = ctx.enter_context(tc.tile_pool(name="ppa", bufs=4, space="PSUM"))
    ppe = ctx.enter_context(tc.tile_pool(name="ppe", bufs=2, space="PSUM"))
    ppb = ctx.enter_context(tc.tile_pool(name="ppb", bufs=2, space="PSUM"))

    # --- phase 0: coalesced DMAs (one 3D-AP transfer per tensor; fewer
    # dma_start instructions and semaphores). z matmuls are first in the PE
    # stream, so w1h/hid go first; then w1e + the first encT chunk.
    w1h_all = wpool.tile([P, KC * D], DT, name="w1h_all")
    nc.sync.dma_start(w1h_all[:].rearrange("p (k m) -> p k m", k=KC),
                      w1h.rearrange("(k p) m -> p k m", p=P))
    hid_all = const.tile([P, KC * BL], DT, name="hid_all")
    nc.sync.dma_start(hid_all[:].rearrange("p (k b) -> p k b", k=KC),
                      hidT.rearrange("(k p) b -> p k b", p=P))
    w1e_all = wpool.tile([P, KC * D], WT, name="w1e_all")
    nc.sync.dma_start(w1e_all[:].rearrange("p (k m) -> p k m", k=KC),
                      w1e.rearrange("(k p) m -> p k m", p=P))
    et0 = epool.tile([P, KC * S_SUB], DT, name="et_big", tag="et_big")
    nc.sync.dma_start(
        et0[:].rearrange("p (k s) -> p k s", k=KC),
        encT[0].rearrange("(k p) s -> p k s", p=P)[:, :, 0:S_SUB])
    b1_all = const.tile([P, KC], F32, name="b1_all")
    nc.sync.dma_start(b1_all[:].rearrange("p (k o) -> p k o", k=KC),
                      b1.rearrange("(k p) o -> p k o", p=P))
    w2_all = const.tile([P, KC * 2], DT, name="w2_all")
    nc.sync.dma_start(w2_all[:].rearrange("p (k c) -> p k c", k=KC),
                      w2.rearrange("(k p) c -> p k c", p=P))
    onesr_t = const.tile([1, P], WT, name="onesr_t")
    nc.sync.dma_start(onesr_t[:], onesr[:])
    w1e_t = [w1e_all[:, k * D:(k + 1) * D] for k in range(KC)]
    w1h_t = [w1h_all[:, k * D:(k + 1) * D] for k in range(KC)]
    hid_t = [hid_all[:, k * BL:(k + 1) * BL] for k in range(KC)]
    b1_t = [b1_all[:, k:k + 1] for k in range(KC)]
    w2_t = [w2_all[:, k * 2:(k + 1) * 2] for k in range(KC)]

    # per-batch bias z = hid @ W1_hid + b1
    z_sb = []
    for m in range(KC):
        zp = ppa.tile([P, BL], F32, name="zp", tag="ppa_t")
        for k in range(KC):
            nc.tensor.matmul(
                zp[:], lhsT=w1h_t[k][:, m * P:(m + 1) * P], rhs=hid_t[k],
                start=(k == 0), stop=(k == KC - 1))
        zt = const.tile([P, BL], F32, name=f"z_{m}")
        nc.vector.tensor_scalar_add(zt[:], zp[:], b1_t[m])
        z_sb.append(zt)

    # per-invocation accumulators for the deferred finalize
    zvec = fpool.tile([1, BL], F32, name="zvec")
    ctxall = fpool.tile([P, BL * KC], F32, name="ctxall")

    # --- pipelined main loop ---
    state = {}    # per-batch: z_parts tile + list of per-chunk ctx tiles
    pending = None  # chunk awaiting exp/broadcast/ctx emission

    def emit_post(pend):
        """exp, p-broadcast, and DVE context work for a finished chunk."""
        pb, pss, e_ps, et_big = pend
        st = state[pb]
        p_row = spool.tile([1, S_SUB], DT, name="p_row", tag="p_row")
        nc.scalar.activation(p_row[:], e_ps[0:1, :], AFT.Exp,
                             accum_out=st["z_parts"][0:1, pss:pss + 1])
        p_bc = ppb.tile([P, S_SUB], F32, name="p_bc")
        nc.tensor.matmul(p_bc[:], lhsT=onesr_t[:], rhs=p_row[:],
                         start=True, stop=True)
        # bf16 SBUF copy of the broadcast so the DVE multiply+reduce run in
        # the all-16-bit all-SBUF 4x perf mode (PSUM f32 operands force 1x)
        pbc_sb = pbpool.tile([P, S_SUB], DT, name="pbc_sb", tag="pbc_sb")
        nc.scalar.activation(pbc_sb[:], p_bc[:], AFT.Copy)
        scratch = spool2.tile([P, KC * S_SUB], DT, name="scr", tag="scr")
        p_rep = pbc_sb[:].rearrange("p (o s) -> p o s",
                                    o=1).broadcast_to((P, KC, S_SUB))
        nc.vector.tensor_tensor(
            scratch[:].rearrange("p (k s) -> p k s", k=KC),
            et_big[:].rearrange("p (k s) -> p k s", k=KC),
            p_rep, ALU.mult)
        cred = cpool.tile([P, KC], DT, name="cred", tag="cred")
        with nc.allow_low_precision(reason="bf16 cred keeps the DVE reduce "
                                    "in 4x mode; ~0.4% on a 2e-2 budget"):
            nc.vector.tensor_reduce(
                cred[:], scratch[:].rearrange("p (k s) -> p k s", k=KC),
                axis=mybir.AxisListType.X, op=ALU.add)
        st["creds"].append(cred)
        if pss == NSS - 1:
            emit_batch_reduce(pb)

    def emit_batch_reduce(pb):
        """Per-batch: Z partial sum + unnormalized ctx into the rep-wide
        accumulators. Runs overlapped with the next batch's matmuls."""
        st = state.pop(pb)
        nc.vector.tensor_reduce(zvec[0:1, pb:pb + 1], st["z_parts"][:],
                                axis=mybir.AxisListType.X, op=ALU.add)
        creds = st["creds"]
        dst = ctxall[:, pb * KC:(pb + 1) * KC]
        nc.vector.tensor_tensor(dst, creds[0][:], creds[1][:], ALU.add)
        nc.vector.tensor_tensor(dst, dst, creds[2][:], ALU.add)
        nc.vector.tensor_tensor(dst, dst, creds[3][:], ALU.add)

    for ci, (b, ss) in enumerate([(b, ss) for b in range(BL)
                                  for ss in range(NSS)]):
        if ss == 0:
            state[b] = {
                "z_parts": spool.tile([1, NSS], F32, name="z_parts",
                                      tag="z_parts"),
                "creds": [],
            }
        if ci == 0:
            et_big = et0
        else:
            et_big = epool.tile([P, KC * S_SUB], DT, name="et_big",
                                tag="et_big")
            nc.sync.dma_start(
                et_big[:].rearrange("p (k s) -> p k s", k=KC),
                encT[b].rearrange("(k p) s -> p k s", p=P)
                [:, :, ss * S_SUB:(ss + 1) * S_SUB])
        e_ps = ppe.tile([2, S_SUB], F32, name="e_ps")
        h_prev = None
        for m in range(KC):
            hp = ppa.tile([P, S_SUB], F32, name="hp", tag="ppa_t")
            for k in range(KC):
                nc.tensor.matmul(
                    hp[:], lhsT=w1e_t[k][:, m * P:(m + 1) * P],
                    rhs=et_big[:, k * S_SUB:(k + 1) * S_SUB],
                    start=(k == 0), stop=(k == KC - 1))
            if m == 0 and pending is not None:
                emit_post(pending)
                pending = None
            h_sb = hpool.tile([P, S_SUB], DT, name="h_sb", tag="h_sb")
            nc.scalar.activation(h_sb[:], hp[:], AFT.Tanh,
                                 bias=z_sb[m][:, b:b + 1],
                                 scale=1.0 / W1E_SCALE)
            if h_prev is not None:
                nc.tensor.matmul(e_ps[:], lhsT=w2_t[m - 1], rhs=h_prev[:],
                                 start=(m == 1), stop=False)
            h_prev = h_sb
        nc.tensor.matmul(e_ps[:], lhsT=w2_t[KC - 1], rhs=h_prev[:],
                         start=False, stop=True)
        pending = (b, ss, e_ps, et_big)
    emit_post(pending)

    # --- deferred finalize: ctx / Z, one contiguous store ---
    zbc = spool.tile([P, BL], F32, name="zbc", tag="zbc")
    nc.gpsimd.partition_broadcast(zbc[:], zvec[:])
    lnz = spool.tile([P, BL], F32, name="lnz", tag="lnz")
    nc.scalar.activation(lnz[:], zbc[:], AFT.Ln)
    zr = spool.tile([P, BL], F32, name="zr", tag="zr")
    nc.scalar.activation(zr[:], lnz[:], AFT.Exp, scale=-1.0)
    ctxout = fpool.tile([P, BL * KC], F32, name="ctxout")
    for b in range(BL):
        nc.vector.tensor_scalar_mul(ctxout[:, b * KC:(b + 1) * KC],
                                    ctxall[:, b * KC:(b + 1) * KC],
                                    zr[:, b:b + 1])
    nc.sync.dma_start(out[:, :], ctxout[:])


def build_program():
    nc = bacc.Bacc("TRN2", target_bir_lowering=False, debug=False,
                   num_devices=NCORES)
    encT = nc.dram_tensor("encT", [BL, D, S], DT, kind="ExternalInput").ap()
    hidT = nc.dram_tensor("hidT", [D, BL], DT, kind="ExternalInput").ap()
    w1e = nc.dram_tensor("w1e", [D, D], WT, kind="ExternalInput").ap()
    w1h = nc.dram_tensor("w1h", [D, D], DT, kind="ExternalInput").ap()
    b1 = nc.dram_tensor("b1", [D, 1], F32, kind="ExternalInput").ap()
    w2 = nc.dram_tensor("w2", [D, 2], DT, kind="ExternalInput").ap()
    onesr = nc.dram_tensor("onesr", [1, P], WT, kind="ExternalInput").ap()
    # ctx in [partition, batch*KC] layout; host permutes back to [BL, D]
    out = nc.dram_tensor("ctx_out", [P, BL * KC], F32,
                         kind="ExternalOutput").ap()
    with tile.TileContext(nc) as tc:
        with ExitStack() as ctx:
            _body(ctx, tc, encT, hidT, w1e, w1h, b1, w2, onesr, out)
    nc.compile()
    return nc


def prep_in_maps(inputs):
    import ml_dtypes
    bf16 = ml_dtypes.bfloat16
    fp8 = ml_dtypes.float8_e3m4
    enc = np.asarray(inputs["encoder_outputs"], dtype=np.float32)
    hid = np.asarray(inputs["hidden_state"], dtype=np.float32)
    W1 = np.asarray(inputs["W1"], dtype=np.float32)
    b1 = np.asarray(inputs["b1"], dtype=np.float32)
    w2 = np.asarray(inputs["w2"], dtype=np.float32)
    encT = np.ascontiguousarray(enc.transpose(0, 2, 1)).astype(bf16)
    w1e = np.clip(np.ascontiguousarray(W1[:D]) * W1E_SCALE,
                  -15.5, 15.5).astype(fp8)
    w1h = np.ascontiguousarray(W1[D:]).astype(bf16)
    b1c = np.ascontiguousarray(b1.reshape(D, 1))
    w2c = np.zeros((D, 2), dtype=bf16)
    w2c[:, 0] = w2.astype(bf16)
    onesr_np = np.ones((1, P), dtype=fp8)
    in_maps = []
    for c in range(NCORES):
        sl = slice(c * BL, (c + 1) * BL)
        in_maps.append({
            "encT": encT[sl],
            "hidT": np.ascontiguousarray(hid[sl].T).astype(bf16),
            "w1e": w1e,
            "w1h": w1h,
            "b1": b1c,
            "w2": w2c,
            "onesr": onesr_np,
        })
    return in_maps


_NC_CACHE = None


def unpack_out(arr):
    """Device layout [P, BL*KC] -> [BL, D]: [p, b*KC+k] = ctx[b, k*P+p]."""
    return (np.asarray(arr, dtype=np.float32).reshape(P, BL, KC)
            .transpose(1, 2, 0).reshape(BL, D))


def kernel(**inputs):
    global _NC_CACHE
    if _NC_CACHE is None:
        _NC_CACHE = build_program()
    nc = _NC_CACHE
    in_maps = prep_in_maps(inputs)
    res = run_bass_kernel_spmd(nc, in_maps, core_ids=list(range(NCORES)))
    out = np.empty((B, D), dtype=np.float32)
    for c in range(NCORES):
        out[c * BL:(c + 1) * BL] = unpack_out(res.results[c]["ctx_out"])
    return out
